# revision 3
# baseline (speedup 1.0000x reference)
"""Cubic B-spline basis expansion on Trainium2, SPMD across 8 NeuronCores.

Problem: xs [131072] f32, B [4,4] f32 (ascending-power coeffs), n=2048, q=3.
Output [131072, 2048] f32: each row i is zeros except 4 contiguous values at
columns first_i..first_i+3 where first_i = floor(xs[i]) (H=1, T0=0), and
value[k] = sum_p (frac + (q-k))^p * B[k,p].

Strategy (data-parallel, no cross-core comms):
  - shard xs / output rows across 8 cores (16384 rows each)
  - the output DRAM buffer arrives pre-zeroed: run_bass_kernel_spmd (under
    axon -> bass2jax.run_bass_via_pjrt) donates zero-initialized buffers as
    the ExternalOutput backing store, an explicitly supported contract
    ("kernels that don't write every element rely on that").  So the kernel
    writes ONLY the 4 nonzeros per row: a 16-byte indirect-DMA scatter per
    row instead of a 128 MiB dense fill.
  - per core: DVE computes first_i = floor(xs), frac, the 4 basis values
    (Horner in the shifted basis: v_k(f) = ((C3*f+C2)*f+C1)*f+C0 with
    C[k,:] precomputed on host from B, folding in the +(q-k) shift), and a
    per-row element offset; then a few large SWDGE indirect DMAs scatter
    the 16-byte value groups.
  - rows are laid out j-major (row = j*128 + p); the J=128 j-columns are
    split into SCATTER_CHUNKS indirect-DMA calls so that each call's
    in-call index ((j % jc)*128 + p)*2048 + first_i stays < 2^24 and is
    therefore exact through the DVE f32 ALU path; the chunk base
    c*jc*128*2048 rides in via element_offset.
"""
import sys

import numpy as np

for _p in ("/opt/trn_rl_repo",):
    if _p not in sys.path:
        sys.path.insert(0, _p)

import concourse.bass as bass
import concourse.mybir as mybir
from concourse.bass_utils import run_bass_kernel_spmd

# Problem constants (hardcoded per contract)
NS = 131072           # total samples
N = 2048              # knots (output columns)
Q = 3                 # spline order
NCORES = 8
R = NS // NCORES      # 16384 rows per core
P = 128               # SBUF partitions
J = R // P            # 128 j-columns per core
SCATTER_CHUNKS = 128  # indirect-DMA calls per core (HW: 1 index/partition)

F32 = mybir.dt.float32
I32 = mybir.dt.int32
ALU = mybir.AluOpType


def _shifted_coeffs(B_np: np.ndarray) -> np.ndarray:
    """C[k,m] so that sum_p B[k,p]*(f+(Q-k))^p == sum_m C[k,m]*f^m."""
    from math import comb

    Bc = np.asarray(B_np, dtype=np.float64)
    C = np.zeros((Q + 1, Q + 1), dtype=np.float64)
    for k in range(Q + 1):
        t = float(Q - k)
        for m in range(Q + 1):
            C[k, m] = sum(
                Bc[k, p] * comb(p, m) * t ** (p - m) for p in range(m, Q + 1)
            )
    return C


def _build(B_np: np.ndarray, iters: int = 1,
           nchunk: int = SCATTER_CHUNKS) -> bass.Bass:
    # iters > 1 repeats the compute + scatter phase (idempotent) inside one
    # NEFF — used only by the timing harness to measure per-iteration HW
    # time as a slope, cancelling dispatch overhead.
    jc = J // nchunk
    assert jc * nchunk == J
    # in-call index must stay < 2^24 for exact f32 integer arithmetic
    assert ((jc - 1) * P + P - 1) * N + N - 1 < (1 << 24)

    C = _shifted_coeffs(B_np)

    nc = bass.Bass("TRN2")
    xs_d = nc.dram_tensor("xs", [P, J], F32, kind="ExternalInput")
    ib_d = nc.dram_tensor("ibase", [P, J], I32, kind="ExternalInput")
    out_d = nc.dram_tensor("out", [R, N], F32, kind="ExternalOutput")

    with (
        nc.sbuf_tensor("xs_t", [P, J], F32) as xs_t,
        nc.sbuf_tensor("ib_t", [P, J], I32) as ib_t,
        nc.sbuf_tensor("fi_f", [P, J], F32) as fi_f,
        nc.sbuf_tensor("gt_t", [P, J], F32) as gt_t,
        nc.sbuf_tensor("frac", [P, J], F32) as frac,
        nc.sbuf_tensor("fi_i", [P, J], I32) as fi_i,
        nc.sbuf_tensor("idx", [P, J], I32) as idx,
        nc.sbuf_tensor("vals", [P, (Q + 1) * J], F32) as vals,
        nc.semaphore("xsem") as xsem,
        nc.semaphore("bsem") as bsem,
        nc.semaphore("csem") as csem,
        nc.semaphore("ssem") as ssem,
        nc.semaphore("vsem") as vsem,
    ):
        with nc.Block() as block:

            @block.sync
            def _(s):
                s.dma_start(out=xs_t[:], in_=xs_d[:]).then_inc(xsem, 16)

            @block.scalar
            def _(s):
                s.dma_start(out=ib_t[:], in_=ib_d[:]).then_inc(bsem, 16)

            @block.vector
            def _(v):
                # DVE ops are chained through vsem: deep engine pipelines mean
                # same-engine RAW hazards still need semaphore sync.
                nv = 0

                def step(inst):
                    nonlocal nv
                    inst.then_inc(vsem, 1)
                    nv += 1

                def fence():
                    v.wait_ge(vsem, nv)

                v.wait_ge(xsem, 16)
                v.wait_ge(bsem, 16)
                # vals[p, 4j+k] = value_k(row j*128+p)
                vv = vals[:].rearrange("p (j k) -> p j k", k=Q + 1)
                for it in range(iters):
                    if it:
                        # don't overwrite vals/idx while prior scatters read
                        v.wait_ge(ssem, 16 * nchunk * it)
                    # first_i = floor(xs) for xs >= 0, robust to any f32->i32
                    # rounding mode: convert, round-trip, subtract 1 where
                    # the round-trip exceeded xs.
                    step(v.tensor_copy(out=fi_i[:], in_=xs_t[:]))
                    fence()
                    step(v.tensor_copy(out=fi_f[:], in_=fi_i[:]))
                    fence()
                    step(v.tensor_tensor(out=gt_t[:], in0=fi_f[:], in1=xs_t[:],
                                         op=ALU.is_gt))
                    fence()
                    step(v.tensor_tensor(out=fi_f[:], in0=fi_f[:], in1=gt_t[:],
                                         op=ALU.subtract))
                    fence()
                    step(v.tensor_tensor(out=frac[:], in0=xs_t[:], in1=fi_f[:],
                                         op=ALU.subtract))
                    step(v.tensor_copy(out=fi_i[:], in_=fi_f[:]))
                    fence()
                    # Horner stages, 4 independent k-chains batched per stage;
                    # idx = ibase + first_i rides in the first batch (exact:
                    # both < 2^24).
                    step(v.tensor_tensor(out=idx[:], in0=ib_t[:], in1=fi_i[:],
                                         op=ALU.add))
                    for k in range(Q + 1):
                        step(v.tensor_scalar(out=vv[:, :, k], in0=frac[:],
                                             scalar1=float(C[k, 3]),
                                             scalar2=float(C[k, 2]),
                                             op0=ALU.mult, op1=ALU.add))
                    fence()
                    for k in range(Q + 1):
                        step(v.tensor_tensor(out=vv[:, :, k], in0=vv[:, :, k],
                                             in1=frac[:], op=ALU.mult))
                    fence()
                    for k in range(Q + 1):
                        step(v.tensor_scalar(out=vv[:, :, k], in0=vv[:, :, k],
                                             scalar1=float(C[k, 1]),
                                             scalar2=None, op0=ALU.add))
                    fence()
                    for k in range(Q + 1):
                        step(v.tensor_tensor(out=vv[:, :, k], in0=vv[:, :, k],
                                             in1=frac[:], op=ALU.mult))
                    fence()
                    for k in range(Q + 1):
                        step(v.tensor_scalar(out=vv[:, :, k], in0=vv[:, :, k],
                                             scalar1=float(C[k, 0]),
                                             scalar2=None, op0=ALU.add))
                    fence()
                    v.sem_inc(csem, 1)

            @block.gpsimd
            def _(g):
                for it in range(iters):
                    g.wait_ge(csem, it + 1)
                    for c in range(nchunk):
                        # each index covers the 4 contiguous values of one
                        # output row; chunk base rides in element_offset
                        g.indirect_dma_start(
                            out=out_d[:],
                            out_offset=bass.IndirectOffsetOnAxis(
                                ap=idx[:, c * jc:(c + 1) * jc], axis=1),
                            in_=vals[:, (Q + 1) * jc * c:(Q + 1) * jc * (c + 1)],
                            in_offset=None,
                            element_offset=c * jc * P * N,
                        ).then_inc(ssem, 16)
                g.wait_ge(ssem, 16 * nchunk * iters)

    return nc


_CACHE: dict[tuple, bass.Bass] = {}


def _get_program(B: np.ndarray) -> bass.Bass:
    key = np.asarray(B, dtype=np.float32).tobytes()
    if key not in _CACHE:
        _CACHE[key] = _build(B)
    return _CACHE[key]


def _in_maps(xs: np.ndarray, nchunk: int = SCATTER_CHUNKS) -> list[dict[str, np.ndarray]]:
    # j-major row layout: xs2d[p, j] = xs_shard[j*P + p]; in-call row base
    # is ((j % jc)*P + p)*N (< 2^24 so DVE f32-ALU int math is exact); the
    # chunk base c*jc*P*N goes in via indirect-DMA element_offset.
    jc = J // nchunk
    jj, pp = np.meshgrid(np.arange(J, dtype=np.int32),
                         np.arange(P, dtype=np.int32))
    ibase = ((jj % jc) * P + pp) * N
    maps = []
    for c in range(NCORES):
        shard = np.asarray(xs[c * R:(c + 1) * R], dtype=np.float32)
        xs2d = np.ascontiguousarray(shard.reshape(J, P).T)
        maps.append({"xs": xs2d, "ibase": np.ascontiguousarray(ibase)})
    return maps


def kernel(xs, B, n, q):
    xs = np.asarray(xs, dtype=np.float32)
    B = np.asarray(B, dtype=np.float32)
    n = int(np.asarray(n)) if not isinstance(n, int) else n
    q = int(np.asarray(q)) if not isinstance(q, int) else q
    assert xs.shape == (NS,), xs.shape
    assert B.shape == (Q + 1, Q + 1), B.shape
    assert n == N and q == Q, (n, q)

    nc = _get_program(B)
    try:
        res = run_bass_kernel_spmd(nc, _in_maps(xs), core_ids=list(range(NCORES)))
    except Exception:
        # one retry for transient device-state errors (e.g. a wedged core
        # left over from a previous process)
        res = run_bass_kernel_spmd(nc, _in_maps(xs), core_ids=list(range(NCORES)))
    return np.concatenate([res.results[c]["out"] for c in range(NCORES)], axis=0)
